# revision 52
# baseline (speedup 1.0000x reference)
"""NetVLAD forward kernel for Trainium2 (8 NeuronCores, data-parallel over batch).

Shapes (hardcoded): x (64, 4096, 128) f32, centroids/weight (64, 128), bias (64),
masks (64, 4096). Output (64, 8192) f32. Each core handles 8 samples.

Math (per sample):
  xn = x / ||x||_row                      (row L2 norm over d)
  logits = xn @ w.T + b ; a = softmax_k(logits) * mask
  vlad[k,d] = sum_c a*xn - (sum_c a) * cent[k,d] ; intra + global L2 norm.

Device algorithm (all matmuls bf16, big-instruction softmax):
  xsq = Square(x) bf16 (ACT); ss = reduce_d(xsq) (DVE); s = exp(-.5 ln ss)
  xn  = x * s_bcast -> bf16 [tokens, d] (+ones aug col)      (DVE+GpSimd)
  xnt = PE-transpose(xn), evacuated by DMA (bf16 PSUM -> SBUF)
  per 16-tile half (double-buffered PSUM):
    pr  = xnt.T @ wt (+ b via 2 bf16 A/B rows ⊗ ones; PE psum accum)
    negM = -max_k pr (DVE) -> PE-transpose -> Mrow; pr += Mrow ⊗ sel (PE)
    g   = Exp(pr) per 512-col bank (ACT, const scale/bias)
    Z   = reduce_k g (DVE); a = g * (mask/Z)_bcast (GpSimd, in place)
    vlad_raw[k, 0:128] (+ colsum col 128) += a.T @ [xn | 1]  (PE)
Epilogue (per core): vlad = first - colsum*cent, intra + global L2 norm.
"""

import numpy as np
import ml_dtypes

import concourse.bass as bass
import concourse.bass_isa as bass_isa
import concourse.mybir as mybir
import concourse.tile as tile
from concourse import bacc
from concourse.bass_utils import run_bass_kernel_spmd

f32 = mybir.dt.float32
bf16 = mybir.dt.bfloat16
AF = mybir.ActivationFunctionType
ALU = mybir.AluOpType

N, C, D, K = 64, 4096, 128, 64
NCORES = 8
NS = N // NCORES          # samples per core
J = C // 128              # 32 token-tiles per sample
TCH = 8                   # transpose tiles per PSUM chunk (1 bank, bf16)
ECH = 8                   # logits tiles per PSUM bank (512 f32)
HCH = 16                  # tiles per half (pr double-buffer unit)
XW = 130                  # xn free width: 128 data + 1 ones-aug (+1 pad)
DVE_XN = 32               # xn tiles computed on DVE (rest on GpSimd)

_CACHE = {}


def _build_nc():
    nc = bacc.Bacc("TRN2", target_bir_lowering=False)
    x_d = nc.dram_tensor("x", [NS, C, D], f32, kind="ExternalInput")
    wt_d = nc.dram_tensor("wt", [D, K], bf16, kind="ExternalInput")
    ab_d = nc.dram_tensor("ab", [2, ECH * K], bf16, kind="ExternalInput")
    sel_d = nc.dram_tensor("sel", [HCH, HCH * K], bf16, kind="ExternalInput")
    cent_d = nc.dram_tensor("cent", [K, D], f32, kind="ExternalInput")
    ident_d = nc.dram_tensor("ident", [128, 128], bf16, kind="ExternalInput")
    mask_d = nc.dram_tensor("masks", [128, NS, J], f32, kind="ExternalInput")
    out_d = nc.dram_tensor("out", [NS, K * D], f32, kind="ExternalOutput")

    with tile.TileContext(nc) as tc:
        _netvlad(tc, x_d, wt_d, ab_d, sel_d, cent_d, ident_d, mask_d, out_d)
    nc.compile()
    return nc


def _netvlad(tc, x_d, wt_d, ab_d, sel_d, cent_d, ident_d, mask_d, out_d):
    nc = tc.nc
    from contextlib import ExitStack

    with ExitStack() as ctx:
        singles = ctx.enter_context(tc.tile_pool(name="singles", bufs=1))
        xpool = ctx.enter_context(tc.tile_pool(name="xp", bufs=2))
        sqpool = ctx.enter_context(tc.tile_pool(name="sqp", bufs=2))
        xnpool = ctx.enter_context(tc.tile_pool(name="xnp", bufs=2))
        xtpool = ctx.enter_context(tc.tile_pool(name="xtp", bufs=2))
        gpool = ctx.enter_context(tc.tile_pool(name="gp", bufs=2))
        stats = ctx.enter_context(tc.tile_pool(name="stats", bufs=2))
        ptpool = ctx.enter_context(tc.tile_pool(name="ptp", bufs=2, space="PSUM"))
        prpool = ctx.enter_context(tc.tile_pool(name="prp", bufs=2, space="PSUM"))
        pmpool = ctx.enter_context(tc.tile_pool(name="pmp", bufs=1, space="PSUM"))
        pvpool = ctx.enter_context(tc.tile_pool(name="pvp", bufs=1, space="PSUM"))

        # ---- constants ----
        wt_s = singles.tile([D, K], bf16)
        nc.sync.dma_start(out=wt_s, in_=wt_d[:, :])
        ab_s = singles.tile([2, ECH * K], bf16)
        nc.sync.dma_start(out=ab_s, in_=ab_d[:, :])
        sel_s = singles.tile([HCH, HCH * K], bf16)
        nc.sync.dma_start(out=sel_s, in_=sel_d[:, :])
        cent_s = singles.tile([K, D], f32)
        nc.sync.dma_start(out=cent_s, in_=cent_d[:, :])
        ident = singles.tile([128, 128], bf16)
        nc.sync.dma_start(out=ident, in_=ident_d[:, :])
        mask_s = singles.tile([128, NS, J], f32)
        nc.sync.dma_start(out=mask_s, in_=mask_d[:, :, :])
        ones2 = singles.tile([2, 128], bf16)
        nc.vector.memset(ones2, 1.0)
        # staging for per-sample vlad rows + colsum (64 partitions)
        vst = singles.tile([K, NS, 129], f32)

        def front_a(n):
            """DMA the sample in and square it (ACT)."""
            x_s = xpool.tile([128, J, D], f32, tag="x", bufs=3)
            nc.sync.dma_start(
                out=x_s, in_=x_d[n, :, :].rearrange("(p j) d -> p j d", j=J)
            )
            xsq = sqpool.tile([128, J, D], bf16, tag="xsq", bufs=3)
            nc.scalar.activation(out=xsq, in_=x_s, func=AF.Square)
            return x_s, xsq

        def front_b(n, x_s, xsq):
            """ss folds -> s -> xn (DVE/GpSimd/ACT)."""
            xf1 = sqpool.tile([128, J, 64], bf16, tag="xf1")
            ss = stats.tile([128, J], bf16, tag="ss")
            with nc.allow_low_precision(reason="ss bf16: 0.4% rel, gate 2e-2"):
                nc.vector.tensor_tensor(
                    out=xf1,
                    in0=xsq[:, :, 0:64],
                    in1=xsq[:, :, 64:128],
                    op=ALU.add,
                )
                xf2 = sqpool.tile([128, J, 32], bf16, tag="xf2")
                nc.vector.tensor_tensor(
                    out=xf2, in0=xf1[:, :, 0:32], in1=xf1[:, :, 32:64], op=ALU.add
                )
                nc.vector.tensor_reduce(
                    out=ss, in_=xf2, axis=mybir.AxisListType.X, op=ALU.add
                )
            rss = stats.tile([128, J], f32, tag="rss")
            nc.vector.reciprocal(out=rss, in_=ss)
            sv = stats.tile([128, J], f32, tag="sv")
            nc.scalar.activation(out=sv, in_=rss, func=AF.Sqrt)
            # s replicated x8 so the multiply's in1 has a contiguous inner
            # run (pure stride-0 broadcast runs DVE at half rate)
            s8 = stats.tile([128, J, 8], f32, tag="s8")
            nc.vector.tensor_copy(
                out=s8, in_=sv.unsqueeze(2).broadcast_to([128, J, 8])
            )
            # xn = x * s (broadcast s along d), bf16, + ones aug col
            xn = xnpool.tile([128, J, XW], bf16, tag="xn")
            nc.vector.memset(xn[:, :, D], 1.0)
            nc.vector.tensor_tensor(
                out=xn[:, 0:DVE_XN, 0:D].rearrange(
                    "p j (u e) -> p j u e", e=8
                ),
                in0=x_s[:, 0:DVE_XN, :].rearrange("p j (u e) -> p j u e", e=8),
                in1=s8[:, 0:DVE_XN, :]
                .unsqueeze(2)
                .broadcast_to([128, DVE_XN, 16, 8]),
                op=ALU.mult,
            )
            if DVE_XN < J:
                nc.gpsimd.tensor_tensor(
                    out=xn[:, DVE_XN:J, 0:D].rearrange(
                        "p j (u e) -> p j u e", e=8
                    ),
                    in0=x_s[:, DVE_XN:J, :].rearrange(
                        "p j (u e) -> p j u e", e=8
                    ),
                    in1=s8[:, DVE_XN:J, :]
                    .unsqueeze(2)
                    .broadcast_to([128, J - DVE_XN, 16, 8]),
                    op=ALU.mult,
                )
            return xn

        def back(n, xn):
            negM = stats.tile([128, J], bf16, tag="negM")
            g = gpool.tile([128, J, K], bf16, tag="g")
            xnt = xtpool.tile([128, J, 128], bf16, tag="xnt")
            pv = pvpool.tile([K, D + 1], f32, tag="pv")

            for h in range(J // HCH):
                # S6/S7: PE transpose xn -> psum (bf16); evacuate via DMA
                for t2 in range(HCH // TCH):
                    jb = h * HCH + t2 * TCH
                    pt = ptpool.tile([128, TCH * 128], bf16, tag="pt")
                    for jj in range(TCH):
                        nc.tensor.transpose(
                            pt[:, jj * 128 : (jj + 1) * 128],
                            xn[:, jb + jj, 0:D],
                            ident,
                        )
                    if t2 == 0:
                        nc.vector.tensor_copy(
                            out=xnt[:, jb : jb + TCH, :],
                            in_=pt.rearrange("p (c d) -> p c d", c=TCH),
                        )
                    else:
                        nc.scalar.copy(
                            out=xnt[:, jb : jb + TCH, :],
                            in_=pt.rearrange("p (c d) -> p c d", c=TCH),
                        )

                # S8: logits pr[tok, HCH*K] = xnt.T @ wt + (A+B) bias rows
                pr = prpool.tile([128, HCH * K], f32, tag="pr")
                for jl in range(HCH):
                    nc.tensor.matmul(
                        pr[:, jl * K : (jl + 1) * K],
                        xnt[:, h * HCH + jl, :],
                        wt_s,
                        start=(jl % ECH == 0),
                        stop=False,
                    )
                for bq in range(HCH // ECH):
                    # closes the bank's group so the M-reduce may read it
                    nc.tensor.matmul(
                        pr[:, bq * ECH * K : (bq + 1) * ECH * K],
                        ones2,
                        ab_s,
                        start=False,
                        stop=True,
                    )
                # S9: negM = -max_k (per token, per tile) in bf16, whole half
                nc.vector.tensor_reduce(
                    out=negM[:, h * HCH : (h + 1) * HCH],
                    in_=pr.rearrange("p (c k) -> p c k", c=HCH),
                    axis=mybir.AxisListType.X,
                    op=ALU.max,
                    negate=True,
                )
                # S10: Mrow[jl, tok] = transpose(negM half) (PE) -> SBUF (ACT)
                pm = pmpool.tile([HCH, 128], bf16, tag="pm")
                nc.tensor.transpose(pm, negM[:, h * HCH : (h + 1) * HCH], ident)
                mrow = stats.tile([HCH, 128], bf16, tag="mrow", bufs=4)
                nc.scalar.copy(out=mrow, in_=pm)
                # S11: pr += Mrow ⊗ sel (per-token max shift, PE)
                for bq in range(HCH // ECH):
                    nc.tensor.matmul(
                        pr[:, bq * ECH * K : (bq + 1) * ECH * K],
                        mrow,
                        sel_s[:, bq * ECH * K : (bq + 1) * ECH * K],
                        start=False,
                        stop=True,
                        skip_group_check=True,
                    )
                # S12: g = Exp(pr) per bank (ACT, const scale/bias)
                for bq in range(HCH // ECH):
                    nc.scalar.activation(
                        out=g[
                            :, h * HCH + bq * ECH : h * HCH + (bq + 1) * ECH, :
                        ],
                        in_=pr[:, bq * ECH * K : (bq + 1) * ECH * K].rearrange(
                            "p (c k) -> p c k", c=ECH
                        ),
                        func=AF.Exp,
                    )
                # S13: Z = sum_k g ; S14: rho = mask / Z   (per half)
                gh = g[:, h * HCH : (h + 1) * HCH, :]
                Zh = stats.tile([128, HCH], bf16, tag="Z", bufs=4)
                gz1 = stats.tile([128, HCH, 32], bf16, tag="gz1", bufs=4)
                with nc.allow_low_precision(reason="Z in [1,64], bf16 0.4%"):
                    nc.vector.tensor_tensor(
                        out=gz1,
                        in0=gh[:, :, 0:32],
                        in1=gh[:, :, 32:64],
                        op=ALU.add,
                    )
                    nc.vector.tensor_reduce(
                        out=Zh, in_=gz1, axis=mybir.AxisListType.X, op=ALU.add
                    )
                zr = stats.tile([128, HCH], f32, tag="zr", bufs=4)
                nc.vector.reciprocal(out=zr, in_=Zh)
                rho = stats.tile([128, HCH], bf16, tag="rho", bufs=4)
                nc.vector.tensor_tensor(
                    out=rho,
                    in0=zr,
                    in1=mask_s[:, n, h * HCH : (h + 1) * HCH],
                    op=ALU.mult,
                )
                # S15: a = g * rho (in place, GpSimd, broadcast rho along k
                # via x8-replicated rho for contiguous inner runs)
                rho8 = stats.tile([128, HCH, 8], bf16, tag="rho8", bufs=4)
                nc.vector.tensor_copy(
                    out=rho8, in_=rho.unsqueeze(2).broadcast_to([128, HCH, 8])
                )
                nc.gpsimd.tensor_tensor(
                    out=gh.rearrange("p j (u e) -> p j u e", e=8),
                    in0=gh.rearrange("p j (u e) -> p j u e", e=8),
                    in1=rho8.unsqueeze(2).broadcast_to([128, HCH, 8, 8]),
                    op=ALU.mult,
                )
                # S16: vlad_raw += a.T @ [xn | 1] (col 128 = colsum(a))
                for jl in range(HCH):
                    j = h * HCH + jl
                    nc.tensor.matmul(
                        pv,
                        g[:, j, :],
                        xn[:, j, 0 : D + 1],
                        start=(j == 0),
                        stop=(j == J - 1),
                    )
            # S17: stage vlad + colsum to SBUF
            nc.scalar.copy(out=vst[:, n, :], in_=pv)

        # ---- skewed pipeline emission: Fa(n+2) | Fb(n+1) | B(n) ----
        fa = {0: front_a(0)}
        if NS > 1:
            fa[1] = front_a(1)
        fb = {0: front_b(0, *fa.pop(0))}
        for n in range(NS):
            if n + 2 < NS:
                fa[n + 2] = front_a(n + 2)
            if n + 1 < NS:
                fb[n + 1] = front_b(n + 1, *fa.pop(n + 1))
            back(n, fb.pop(n))

        # ---- epilogue over all samples: [64, NS, *] ----
        negcs = stats.tile([K, NS], f32, tag="negcs")
        nc.vector.tensor_scalar(
            out=negcs, in0=vst[:, :, 128], scalar1=-1.0, scalar2=None, op0=ALU.mult
        )
        vl = singles.tile([K, NS, D], f32)
        for n in range(NS):
            # vlad = first_term - colsum*cent
            nc.vector.scalar_tensor_tensor(
                out=vl[:, n, :],
                in0=cent_s,
                scalar=negcs[:, n : n + 1],
                in1=vst[:, n, 0:D],
                op0=ALU.mult,
                op1=ALU.add,
            )
        v2 = singles.tile([K, NS, D], f32)
        nc.vector.tensor_tensor(out=v2, in0=vl, in1=vl, op=ALU.mult)
        ssv = stats.tile([K, NS], f32, tag="ssv")
        nc.vector.tensor_reduce(
            out=ssv, in_=v2, axis=mybir.AxisListType.X, op=ALU.add
        )
        # rv = 1/max(||row||, 1e-12)  (clamp ss at 1e-24; recip + sqrt)
        nc.vector.tensor_scalar(
            out=ssv, in0=ssv, scalar1=1e-24, scalar2=None, op0=ALU.max
        )
        rsv = stats.tile([K, NS], f32, tag="rsv")
        nc.vector.reciprocal(out=rsv, in_=ssv)
        rv = stats.tile([K, NS], f32, tag="rv")
        nc.scalar.activation(out=rv, in_=rsv, func=AF.Sqrt)
        # global: gs[n] = sum_k ssv*rv^2 (cross-partition on GpSimd)
        u1 = stats.tile([K, NS], f32, tag="u1")
        nc.vector.tensor_tensor(out=u1, in0=ssv, in1=rv, op=ALU.mult)
        nc.vector.tensor_tensor(out=u1, in0=u1, in1=rv, op=ALU.mult)
        gs = stats.tile([K, NS], f32, tag="gs")
        nc.gpsimd.partition_all_reduce(
            gs, u1, channels=K, reduce_op=bass_isa.ReduceOp.add
        )
        nc.vector.tensor_scalar(
            out=gs, in0=gs, scalar1=1e-24, scalar2=None, op0=ALU.max
        )
        rgs = stats.tile([K, NS], f32, tag="rgs")
        nc.vector.reciprocal(out=rgs, in_=gs)
        rg = stats.tile([K, NS], f32, tag="rg")
        nc.scalar.activation(out=rg, in_=rgs, func=AF.Sqrt)
        fsc = stats.tile([K, NS], f32, tag="fsc")
        nc.vector.tensor_tensor(out=fsc, in0=rv, in1=rg, op=ALU.mult)
        vo = singles.tile([K, NS, D], f32)
        for n in range(NS):
            nc.vector.tensor_scalar(
                out=vo[:, n, :],
                in0=vl[:, n, :],
                scalar1=fsc[:, n : n + 1],
                scalar2=None,
                op0=ALU.mult,
            )
        # one DMA out: [k, n, d] -> out[n, (k d)]
        nc.sync.dma_start(
            out=out_d[:, :].rearrange("n (k d) -> k n d", k=K), in_=vo
        )


def kernel(x, centroids, weight, bias, masks):
    x = np.ascontiguousarray(x, dtype=np.float32)
    centroids = np.asarray(centroids, dtype=np.float32)
    weight = np.asarray(weight, dtype=np.float32)
    bias = np.asarray(bias, dtype=np.float32)
    masks = np.ascontiguousarray(masks, dtype=np.float32)

    if "nc" not in _CACHE:
        _CACHE["nc"] = _build_nc()
    nc = _CACHE["nc"]

    wt = np.ascontiguousarray(weight.T).astype(ml_dtypes.bfloat16)  # [D, K]
    # exact bias fold: lnE = b - max b + 60 split into bf16 A + bf16 B
    lnE = (bias - bias.max() + 60.0).astype(np.float32)
    A = lnE.astype(ml_dtypes.bfloat16)
    B = (lnE - A.astype(np.float32)).astype(ml_dtypes.bfloat16)
    ab = np.stack([np.tile(A, ECH), np.tile(B, ECH)])  # [2, ECH*K]
    ab = np.ascontiguousarray(ab)
    sel = np.zeros((HCH, HCH * K), dtype=ml_dtypes.bfloat16)
    for j in range(HCH):
        sel[j, j * K : (j + 1) * K] = 1.0
    ident = np.eye(128, dtype=np.float32).astype(ml_dtypes.bfloat16)

    in_maps = []
    for c in range(NCORES):
        sl = slice(c * NS, (c + 1) * NS)
        mcore = masks[sl].reshape(NS, 128, J).transpose(1, 0, 2)  # [128, NS, J]
        in_maps.append(
            {
                "x": x[sl],
                "wt": wt,
                "ab": ab,
                "sel": sel,
                "cent": centroids,
                "ident": ident,
                "masks": np.ascontiguousarray(mcore),
            }
        )

    res = run_bass_kernel_spmd(nc, in_maps, core_ids=list(range(NCORES)))
    _CACHE["last_res"] = res
    outs = [res.results[c]["out"] for c in range(NCORES)]
    return np.concatenate(outs, axis=0).reshape(N, K * D).astype(np.float32)


# revision 53
# speedup vs baseline: 1.2344x; 1.2344x over previous
"""NetVLAD forward kernel for Trainium2 (8 NeuronCores, data-parallel over batch).

Shapes (hardcoded): x (64, 4096, 128) f32, centroids/weight (64, 128), bias (64),
masks (64, 4096). Output (64, 8192) f32. Each core handles 8 samples.

Math (per sample):
  xn = x / ||x||_row                      (row L2 norm over d)
  logits = xn @ w.T + b ; a = softmax_k(logits) * mask
  vlad[k,d] = sum_c a*xn - (sum_c a) * cent[k,d] ; intra + global L2 norm.

Device algorithm (all matmuls bf16, big-instruction softmax):
  xsq = Square(x) bf16 (ACT); ss = reduce_d(xsq) (DVE); s = exp(-.5 ln ss)
  xn  = x * s_bcast -> bf16 [tokens, d] (+ones aug col)      (DVE+GpSimd)
  xnt = PE-transpose(xn), evacuated by DMA (bf16 PSUM -> SBUF)
  per 16-tile half (double-buffered PSUM):
    pr  = xnt.T @ wt (+ b via 2 bf16 A/B rows ⊗ ones; PE psum accum)
    negM = -max_k pr (DVE) -> PE-transpose -> Mrow; pr += Mrow ⊗ sel (PE)
    g   = Exp(pr) per 512-col bank (ACT, const scale/bias)
    Z   = reduce_k g (DVE); a = g * (mask/Z)_bcast (GpSimd, in place)
    vlad_raw[k, 0:128] (+ colsum col 128) += a.T @ [xn | 1]  (PE)
Epilogue (per core): vlad = first - colsum*cent, intra + global L2 norm.
"""

import numpy as np
import ml_dtypes

import concourse.bass as bass
import concourse.bass_isa as bass_isa
import concourse.mybir as mybir
import concourse.tile as tile
from concourse import bacc
from concourse.bass_utils import run_bass_kernel_spmd

f32 = mybir.dt.float32
bf16 = mybir.dt.bfloat16
AF = mybir.ActivationFunctionType
ALU = mybir.AluOpType

N, C, D, K = 64, 4096, 128, 64
NCORES = 8
NS = N // NCORES          # samples per core
J = C // 128              # 32 token-tiles per sample
TCH = 8                   # transpose tiles per PSUM chunk (1 bank, bf16)
ECH = 8                   # logits tiles per PSUM bank (512 f32)
HCH = 16                  # tiles per half (pr double-buffer unit)
XW = 130                  # xn free width: 128 data + 1 ones-aug (+1 pad)
DVE_XN = 32               # xn tiles computed on DVE (rest on GpSimd)

_CACHE = {}


def _build_nc():
    nc = bacc.Bacc("TRN2", target_bir_lowering=False)
    x_d = nc.dram_tensor("x", [NS, C, D], f32, kind="ExternalInput")
    wt_d = nc.dram_tensor("wt", [D, K], bf16, kind="ExternalInput")
    ab_d = nc.dram_tensor("ab", [2, ECH * K], bf16, kind="ExternalInput")
    sel_d = nc.dram_tensor("sel", [HCH, HCH * K], bf16, kind="ExternalInput")
    cent_d = nc.dram_tensor("cent", [K, D], f32, kind="ExternalInput")
    ident_d = nc.dram_tensor("ident", [128, 128], bf16, kind="ExternalInput")
    mask_d = nc.dram_tensor("masks", [128, NS, J], f32, kind="ExternalInput")
    out_d = nc.dram_tensor("out", [NS, K * D], f32, kind="ExternalOutput")

    with tile.TileContext(nc) as tc:
        _netvlad(tc, x_d, wt_d, ab_d, sel_d, cent_d, ident_d, mask_d, out_d)
    nc.compile()
    return nc


def _netvlad(tc, x_d, wt_d, ab_d, sel_d, cent_d, ident_d, mask_d, out_d):
    nc = tc.nc
    from contextlib import ExitStack

    with ExitStack() as ctx:
        singles = ctx.enter_context(tc.tile_pool(name="singles", bufs=1))
        xpool = ctx.enter_context(tc.tile_pool(name="xp", bufs=2))
        sqpool = ctx.enter_context(tc.tile_pool(name="sqp", bufs=2))
        xnpool = ctx.enter_context(tc.tile_pool(name="xnp", bufs=2))
        xtpool = ctx.enter_context(tc.tile_pool(name="xtp", bufs=2))
        gpool = ctx.enter_context(tc.tile_pool(name="gp", bufs=2))
        stats = ctx.enter_context(tc.tile_pool(name="stats", bufs=2))
        ptpool = ctx.enter_context(tc.tile_pool(name="ptp", bufs=2, space="PSUM"))
        prpool = ctx.enter_context(tc.tile_pool(name="prp", bufs=2, space="PSUM"))
        pmpool = ctx.enter_context(tc.tile_pool(name="pmp", bufs=1, space="PSUM"))
        pvpool = ctx.enter_context(tc.tile_pool(name="pvp", bufs=1, space="PSUM"))

        # ---- constants ----
        wt_s = singles.tile([D, K], bf16)
        nc.sync.dma_start(out=wt_s, in_=wt_d[:, :])
        ab_s = singles.tile([2, ECH * K], bf16)
        nc.sync.dma_start(out=ab_s, in_=ab_d[:, :])
        sel_s = singles.tile([HCH, HCH * K], bf16)
        nc.sync.dma_start(out=sel_s, in_=sel_d[:, :])
        cent_s = singles.tile([K, D], f32)
        nc.sync.dma_start(out=cent_s, in_=cent_d[:, :])
        ident = singles.tile([128, 128], bf16)
        nc.sync.dma_start(out=ident, in_=ident_d[:, :])
        mask_s = singles.tile([128, NS, J], f32)
        nc.sync.dma_start(out=mask_s, in_=mask_d[:, :, :])
        ones2 = singles.tile([2, 128], bf16)
        nc.vector.memset(ones2, 1.0)
        # staging for per-sample vlad rows + colsum (64 partitions)
        vst = singles.tile([K, NS, 129], f32)

        def front_a(n):
            """DMA the sample in and square it (ACT)."""
            x_s = xpool.tile([128, J, D], f32, tag="x", bufs=3)
            nc.sync.dma_start(
                out=x_s, in_=x_d[n, :, :].rearrange("(p j) d -> p j d", j=J)
            )
            xsq = sqpool.tile([128, J, D], bf16, tag="xsq", bufs=3)
            nc.scalar.activation(out=xsq, in_=x_s, func=AF.Square)
            return x_s, xsq

        def front_b(n, x_s, xsq):
            """ss folds -> s -> xn (DVE/GpSimd/ACT)."""
            xf1 = sqpool.tile([128, J, 64], bf16, tag="xf1")
            ss = stats.tile([128, J], bf16, tag="ss")
            with nc.allow_low_precision(reason="ss bf16: 0.4% rel, gate 2e-2"):
                nc.vector.tensor_tensor(
                    out=xf1,
                    in0=xsq[:, :, 0:64],
                    in1=xsq[:, :, 64:128],
                    op=ALU.add,
                )
                xf2 = sqpool.tile([128, J, 32], bf16, tag="xf2")
                nc.vector.tensor_tensor(
                    out=xf2, in0=xf1[:, :, 0:32], in1=xf1[:, :, 32:64], op=ALU.add
                )
                nc.vector.tensor_reduce(
                    out=ss, in_=xf2, axis=mybir.AxisListType.X, op=ALU.add
                )
            rss = stats.tile([128, J], f32, tag="rss")
            nc.vector.reciprocal(out=rss, in_=ss)
            sv = stats.tile([128, J], f32, tag="sv")
            nc.scalar.activation(out=sv, in_=rss, func=AF.Sqrt)
            # xn = x * s (broadcast s along d), bf16, + ones aug col
            xn = xnpool.tile([128, J, XW], bf16, tag="xn")
            nc.vector.memset(xn[:, :, D], 1.0)
            nc.vector.tensor_tensor(
                out=xn[:, 0:DVE_XN, 0:D],
                in0=x_s[:, 0:DVE_XN, :],
                in1=sv[:, 0:DVE_XN].unsqueeze(2).broadcast_to([128, DVE_XN, D]),
                op=ALU.mult,
            )
            if DVE_XN < J:
                nc.gpsimd.tensor_tensor(
                    out=xn[:, DVE_XN:J, 0:D],
                    in0=x_s[:, DVE_XN:J, :],
                    in1=sv[:, DVE_XN:J]
                    .unsqueeze(2)
                    .broadcast_to([128, J - DVE_XN, D]),
                    op=ALU.mult,
                )
            return xn

        def back(n, xn):
            negM = stats.tile([128, J], bf16, tag="negM")
            g = gpool.tile([128, J, K], bf16, tag="g")
            xnt = xtpool.tile([128, J, 128], bf16, tag="xnt")
            pv = pvpool.tile([K, D + 1], f32, tag="pv")

            for h in range(J // HCH):
                # S6/S7: PE transpose xn -> psum (bf16); evacuate via DMA
                for t2 in range(HCH // TCH):
                    jb = h * HCH + t2 * TCH
                    pt = ptpool.tile([128, TCH * 128], bf16, tag="pt")
                    for jj in range(TCH):
                        nc.tensor.transpose(
                            pt[:, jj * 128 : (jj + 1) * 128],
                            xn[:, jb + jj, 0:D],
                            ident,
                        )
                    if t2 == 0:
                        nc.vector.tensor_copy(
                            out=xnt[:, jb : jb + TCH, :],
                            in_=pt.rearrange("p (c d) -> p c d", c=TCH),
                        )
                    else:
                        nc.scalar.copy(
                            out=xnt[:, jb : jb + TCH, :],
                            in_=pt.rearrange("p (c d) -> p c d", c=TCH),
                        )

                # S8: logits pr[tok, HCH*K] = xnt.T @ wt + (A+B) bias rows
                pr = prpool.tile([128, HCH * K], f32, tag="pr")
                for jl in range(HCH):
                    nc.tensor.matmul(
                        pr[:, jl * K : (jl + 1) * K],
                        xnt[:, h * HCH + jl, :],
                        wt_s,
                        start=(jl % ECH == 0),
                        stop=False,
                    )
                for bq in range(HCH // ECH):
                    # closes the bank's group so the M-reduce may read it
                    nc.tensor.matmul(
                        pr[:, bq * ECH * K : (bq + 1) * ECH * K],
                        ones2,
                        ab_s,
                        start=False,
                        stop=True,
                    )
                # S9: negM = -max_k (per token, per tile) in bf16, whole half
                nc.vector.tensor_reduce(
                    out=negM[:, h * HCH : (h + 1) * HCH],
                    in_=pr.rearrange("p (c k) -> p c k", c=HCH),
                    axis=mybir.AxisListType.X,
                    op=ALU.max,
                    negate=True,
                )
                # S10: Mrow[jl, tok] = transpose(negM half) (PE) -> SBUF (ACT)
                pm = pmpool.tile([HCH, 128], bf16, tag="pm")
                nc.tensor.transpose(pm, negM[:, h * HCH : (h + 1) * HCH], ident)
                mrow = stats.tile([HCH, 128], bf16, tag="mrow", bufs=4)
                nc.scalar.copy(out=mrow, in_=pm)
                # S11: pr += Mrow ⊗ sel (per-token max shift, PE)
                for bq in range(HCH // ECH):
                    nc.tensor.matmul(
                        pr[:, bq * ECH * K : (bq + 1) * ECH * K],
                        mrow,
                        sel_s[:, bq * ECH * K : (bq + 1) * ECH * K],
                        start=False,
                        stop=True,
                        skip_group_check=True,
                    )
                # S12: g = Exp(pr) per bank (ACT, const scale/bias)
                for bq in range(HCH // ECH):
                    nc.scalar.activation(
                        out=g[
                            :, h * HCH + bq * ECH : h * HCH + (bq + 1) * ECH, :
                        ],
                        in_=pr[:, bq * ECH * K : (bq + 1) * ECH * K].rearrange(
                            "p (c k) -> p c k", c=ECH
                        ),
                        func=AF.Exp,
                    )
                # S13: Z = sum_k g ; S14: rho = mask / Z   (per half)
                gh = g[:, h * HCH : (h + 1) * HCH, :]
                Zh = stats.tile([128, HCH], bf16, tag="Z", bufs=4)
                gz1 = stats.tile([128, HCH, 32], bf16, tag="gz1", bufs=4)
                with nc.allow_low_precision(reason="Z in [1,64], bf16 0.4%"):
                    nc.vector.tensor_tensor(
                        out=gz1,
                        in0=gh[:, :, 0:32],
                        in1=gh[:, :, 32:64],
                        op=ALU.add,
                    )
                    nc.vector.tensor_reduce(
                        out=Zh, in_=gz1, axis=mybir.AxisListType.X, op=ALU.add
                    )
                zr = stats.tile([128, HCH], f32, tag="zr", bufs=4)
                nc.vector.reciprocal(out=zr, in_=Zh)
                rho = stats.tile([128, HCH], bf16, tag="rho", bufs=4)
                nc.vector.tensor_tensor(
                    out=rho,
                    in0=zr,
                    in1=mask_s[:, n, h * HCH : (h + 1) * HCH],
                    op=ALU.mult,
                )
                # S15: a = g * rho (in place, GpSimd, broadcast rho along k)
                nc.gpsimd.tensor_tensor(
                    out=gh,
                    in0=gh,
                    in1=rho.unsqueeze(2).broadcast_to([128, HCH, K]),
                    op=ALU.mult,
                )
                # S16: vlad_raw += a.T @ [xn | 1] (col 128 = colsum(a))
                for jl in range(HCH):
                    j = h * HCH + jl
                    nc.tensor.matmul(
                        pv,
                        g[:, j, :],
                        xn[:, j, 0 : D + 1],
                        start=(j == 0),
                        stop=(j == J - 1),
                    )
            # S17: stage vlad + colsum to SBUF
            nc.scalar.copy(out=vst[:, n, :], in_=pv)

        # ---- skewed pipeline emission: Fa(n+2) | Fb(n+1) | B(n) ----
        fa = {0: front_a(0)}
        if NS > 1:
            fa[1] = front_a(1)
        fb = {0: front_b(0, *fa.pop(0))}
        for n in range(NS):
            if n + 2 < NS:
                fa[n + 2] = front_a(n + 2)
            if n + 1 < NS:
                fb[n + 1] = front_b(n + 1, *fa.pop(n + 1))
            back(n, fb.pop(n))

        # ---- epilogue over all samples: [64, NS, *] ----
        negcs = stats.tile([K, NS], f32, tag="negcs")
        nc.vector.tensor_scalar(
            out=negcs, in0=vst[:, :, 128], scalar1=-1.0, scalar2=None, op0=ALU.mult
        )
        vl = singles.tile([K, NS, D], f32)
        for n in range(NS):
            # vlad = first_term - colsum*cent
            nc.vector.scalar_tensor_tensor(
                out=vl[:, n, :],
                in0=cent_s,
                scalar=negcs[:, n : n + 1],
                in1=vst[:, n, 0:D],
                op0=ALU.mult,
                op1=ALU.add,
            )
        v2 = singles.tile([K, NS, D], f32)
        nc.vector.tensor_tensor(out=v2, in0=vl, in1=vl, op=ALU.mult)
        ssv = stats.tile([K, NS], f32, tag="ssv")
        nc.vector.tensor_reduce(
            out=ssv, in_=v2, axis=mybir.AxisListType.X, op=ALU.add
        )
        # rv = 1/max(||row||, 1e-12)  (clamp ss at 1e-24; recip + sqrt)
        nc.vector.tensor_scalar(
            out=ssv, in0=ssv, scalar1=1e-24, scalar2=None, op0=ALU.max
        )
        rsv = stats.tile([K, NS], f32, tag="rsv")
        nc.vector.reciprocal(out=rsv, in_=ssv)
        rv = stats.tile([K, NS], f32, tag="rv")
        nc.scalar.activation(out=rv, in_=rsv, func=AF.Sqrt)
        # global: gs[n] = sum_k ssv*rv^2 (cross-partition on GpSimd)
        u1 = stats.tile([K, NS], f32, tag="u1")
        nc.vector.tensor_tensor(out=u1, in0=ssv, in1=rv, op=ALU.mult)
        nc.vector.tensor_tensor(out=u1, in0=u1, in1=rv, op=ALU.mult)
        gs = stats.tile([K, NS], f32, tag="gs")
        nc.gpsimd.partition_all_reduce(
            gs, u1, channels=K, reduce_op=bass_isa.ReduceOp.add
        )
        nc.vector.tensor_scalar(
            out=gs, in0=gs, scalar1=1e-24, scalar2=None, op0=ALU.max
        )
        rgs = stats.tile([K, NS], f32, tag="rgs")
        nc.vector.reciprocal(out=rgs, in_=gs)
        rg = stats.tile([K, NS], f32, tag="rg")
        nc.scalar.activation(out=rg, in_=rgs, func=AF.Sqrt)
        fsc = stats.tile([K, NS], f32, tag="fsc")
        nc.vector.tensor_tensor(out=fsc, in0=rv, in1=rg, op=ALU.mult)
        vo = singles.tile([K, NS, D], f32)
        for n in range(NS):
            nc.vector.tensor_scalar(
                out=vo[:, n, :],
                in0=vl[:, n, :],
                scalar1=fsc[:, n : n + 1],
                scalar2=None,
                op0=ALU.mult,
            )
        # one DMA out: [k, n, d] -> out[n, (k d)]
        nc.sync.dma_start(
            out=out_d[:, :].rearrange("n (k d) -> k n d", k=K), in_=vo
        )


def kernel(x, centroids, weight, bias, masks):
    x = np.ascontiguousarray(x, dtype=np.float32)
    centroids = np.asarray(centroids, dtype=np.float32)
    weight = np.asarray(weight, dtype=np.float32)
    bias = np.asarray(bias, dtype=np.float32)
    masks = np.ascontiguousarray(masks, dtype=np.float32)

    if "nc" not in _CACHE:
        _CACHE["nc"] = _build_nc()
    nc = _CACHE["nc"]

    wt = np.ascontiguousarray(weight.T).astype(ml_dtypes.bfloat16)  # [D, K]
    # exact bias fold: lnE = b - max b + 60 split into bf16 A + bf16 B
    lnE = (bias - bias.max() + 60.0).astype(np.float32)
    A = lnE.astype(ml_dtypes.bfloat16)
    B = (lnE - A.astype(np.float32)).astype(ml_dtypes.bfloat16)
    ab = np.stack([np.tile(A, ECH), np.tile(B, ECH)])  # [2, ECH*K]
    ab = np.ascontiguousarray(ab)
    sel = np.zeros((HCH, HCH * K), dtype=ml_dtypes.bfloat16)
    for j in range(HCH):
        sel[j, j * K : (j + 1) * K] = 1.0
    ident = np.eye(128, dtype=np.float32).astype(ml_dtypes.bfloat16)

    in_maps = []
    for c in range(NCORES):
        sl = slice(c * NS, (c + 1) * NS)
        mcore = masks[sl].reshape(NS, 128, J).transpose(1, 0, 2)  # [128, NS, J]
        in_maps.append(
            {
                "x": x[sl],
                "wt": wt,
                "ab": ab,
                "sel": sel,
                "cent": centroids,
                "ident": ident,
                "masks": np.ascontiguousarray(mcore),
            }
        )

    res = run_bass_kernel_spmd(nc, in_maps, core_ids=list(range(NCORES)))
    _CACHE["last_res"] = res
    outs = [res.results[c]["out"] for c in range(NCORES)]
    return np.concatenate(outs, axis=0).reshape(N, K * D).astype(np.float32)


# revision 54
# speedup vs baseline: 1.2470x; 1.0102x over previous
"""NetVLAD forward kernel for Trainium2 (8 NeuronCores, data-parallel over batch).

Shapes (hardcoded): x (64, 4096, 128) f32, centroids/weight (64, 128), bias (64),
masks (64, 4096). Output (64, 8192) f32. Each core handles 8 samples.

Math (per sample):
  xn = x / ||x||_row                      (row L2 norm over d)
  logits = xn @ w.T + b ; a = softmax_k(logits) * mask
  vlad[k,d] = sum_c a*xn - (sum_c a) * cent[k,d] ; intra + global L2 norm.

Device algorithm (all matmuls bf16, big-instruction softmax), emitted as a
3-stage software pipeline Fa(n+2) | Fb(n+1) | B(n) to avoid per-engine
head-of-line blocking across samples:
  Fa: DMA x; xsq = Square(x) bf16 (ACT)
  Fb: ss via bf16 pairwise folds + reduce (DVE); s = sqrt(1/ss);
      xn = x * s_bcast -> bf16 [tokens, d] (+ones aug col)  (DVE)
  B, per 16-tile half (double-buffered 2-bank PSUM):
    xnt = PE-transpose(xn) -> bf16 PSUM, evacuated by DVE/ACT copies
    pr  = xnt.T @ wt (+ exact bias via 2 bf16 A/B rows ⊗ ones; PE accum,
          one accumulation group per 2KB PSUM bank)
    negM = -max_k pr (DVE) -> PE-transpose -> Mrow; pr += Mrow ⊗ sel (PE)
    g   = Exp(pr) per 512-col bank (ACT, const scale/bias)
    Z   = reduce_k g (DVE); a = g * (mask/Z)_bcast (GpSimd, in place)
    vlad_raw[k, 0:128] (+ colsum col 128) += a.T @ [xn | 1]  (PE)
Epilogue (per core): vlad = first - colsum*cent, intra + global L2 norm.
"""

import numpy as np
import ml_dtypes

import concourse.bass as bass
import concourse.bass_isa as bass_isa
import concourse.mybir as mybir
import concourse.tile as tile
from concourse import bacc
from concourse.bass_utils import run_bass_kernel_spmd

f32 = mybir.dt.float32
bf16 = mybir.dt.bfloat16
AF = mybir.ActivationFunctionType
ALU = mybir.AluOpType

N, C, D, K = 64, 4096, 128, 64
NCORES = 8
NS = N // NCORES          # samples per core
J = C // 128              # 32 token-tiles per sample
TCH = 8                   # transpose tiles per PSUM chunk (1 bank, bf16)
ECH = 8                   # logits tiles per PSUM bank (512 f32)
HCH = 16                  # tiles per half (pr double-buffer unit)
XW = 130                  # xn free width: 128 data + 1 ones-aug (+1 pad)
DVE_XN = 32               # xn tiles computed on DVE (rest on GpSimd)

_CACHE = {}


def _build_nc():
    nc = bacc.Bacc("TRN2", target_bir_lowering=False)
    x_d = nc.dram_tensor("x", [NS, C, D], f32, kind="ExternalInput")
    wt_d = nc.dram_tensor("wt", [D, K], bf16, kind="ExternalInput")
    ab_d = nc.dram_tensor("ab", [2, ECH * K], bf16, kind="ExternalInput")
    sel_d = nc.dram_tensor("sel", [HCH, HCH * K], bf16, kind="ExternalInput")
    cent_d = nc.dram_tensor("cent", [K, D], f32, kind="ExternalInput")
    ident_d = nc.dram_tensor("ident", [128, 128], bf16, kind="ExternalInput")
    mask_d = nc.dram_tensor("masks", [128, NS, J], f32, kind="ExternalInput")
    out_d = nc.dram_tensor("out", [NS, K * D], f32, kind="ExternalOutput")

    with tile.TileContext(nc) as tc:
        _netvlad(tc, x_d, wt_d, ab_d, sel_d, cent_d, ident_d, mask_d, out_d)
    nc.compile()
    return nc


def _netvlad(tc, x_d, wt_d, ab_d, sel_d, cent_d, ident_d, mask_d, out_d):
    nc = tc.nc
    from contextlib import ExitStack

    with ExitStack() as ctx:
        singles = ctx.enter_context(tc.tile_pool(name="singles", bufs=1))
        xpool = ctx.enter_context(tc.tile_pool(name="xp", bufs=2))
        sqpool = ctx.enter_context(tc.tile_pool(name="sqp", bufs=2))
        xnpool = ctx.enter_context(tc.tile_pool(name="xnp", bufs=2))
        xtpool = ctx.enter_context(tc.tile_pool(name="xtp", bufs=2))
        gpool = ctx.enter_context(tc.tile_pool(name="gp", bufs=2))
        stats = ctx.enter_context(tc.tile_pool(name="stats", bufs=2))
        ptpool = ctx.enter_context(tc.tile_pool(name="ptp", bufs=2, space="PSUM"))
        prpool = ctx.enter_context(tc.tile_pool(name="prp", bufs=2, space="PSUM"))
        pmpool = ctx.enter_context(tc.tile_pool(name="pmp", bufs=1, space="PSUM"))
        pvpool = ctx.enter_context(tc.tile_pool(name="pvp", bufs=1, space="PSUM"))

        # ---- constants ----
        wt_s = singles.tile([D, K], bf16)
        nc.sync.dma_start(out=wt_s, in_=wt_d[:, :])
        ab_s = singles.tile([2, ECH * K], bf16)
        nc.sync.dma_start(out=ab_s, in_=ab_d[:, :])
        sel_s = singles.tile([HCH, HCH * K], bf16)
        nc.sync.dma_start(out=sel_s, in_=sel_d[:, :])
        cent_s = singles.tile([K, D], f32)
        nc.sync.dma_start(out=cent_s, in_=cent_d[:, :])
        ident = singles.tile([128, 128], bf16)
        nc.sync.dma_start(out=ident, in_=ident_d[:, :])
        mask_s = singles.tile([128, NS, J], f32)
        nc.sync.dma_start(out=mask_s, in_=mask_d[:, :, :])
        ones2 = singles.tile([2, 128], bf16)
        nc.vector.memset(ones2, 1.0)
        # staging for per-sample vlad rows + colsum (64 partitions)
        vst = singles.tile([K, NS, 129], f32)

        def front_a(n):
            """DMA the sample in and square it (ACT)."""
            x_s = xpool.tile([128, J, D], f32, tag="x", bufs=3)
            nc.sync.dma_start(
                out=x_s, in_=x_d[n, :, :].rearrange("(p j) d -> p j d", j=J)
            )
            xsq = sqpool.tile([128, J, D], bf16, tag="xsq", bufs=3)
            nc.scalar.activation(out=xsq, in_=x_s, func=AF.Square)
            return x_s, xsq

        def front_b(n, x_s, xsq):
            """ss folds -> s -> xn (DVE/GpSimd/ACT)."""
            xf1 = sqpool.tile([128, J, 64], bf16, tag="xf1")
            ss = stats.tile([128, J], bf16, tag="ss")
            with nc.allow_low_precision(reason="ss bf16: 0.4% rel, gate 2e-2"):
                nc.vector.tensor_tensor(
                    out=xf1,
                    in0=xsq[:, :, 0:64],
                    in1=xsq[:, :, 64:128],
                    op=ALU.add,
                )
                xf2 = sqpool.tile([128, J, 32], bf16, tag="xf2")
                nc.vector.tensor_tensor(
                    out=xf2, in0=xf1[:, :, 0:32], in1=xf1[:, :, 32:64], op=ALU.add
                )
                nc.vector.tensor_reduce(
                    out=ss, in_=xf2, axis=mybir.AxisListType.X, op=ALU.add
                )
            rss = stats.tile([128, J], f32, tag="rss")
            nc.vector.reciprocal(out=rss, in_=ss)
            sv = stats.tile([128, J], f32, tag="sv")
            nc.scalar.activation(out=sv, in_=rss, func=AF.Sqrt)
            # xn = x * s (broadcast s along d), bf16, + ones aug col
            xn = xnpool.tile([128, J, XW], bf16, tag="xn")
            nc.vector.memset(xn[:, :, D], 1.0)
            nc.vector.tensor_tensor(
                out=xn[:, 0:DVE_XN, 0:D],
                in0=x_s[:, 0:DVE_XN, :],
                in1=sv[:, 0:DVE_XN].unsqueeze(2).broadcast_to([128, DVE_XN, D]),
                op=ALU.mult,
            )
            if DVE_XN < J:
                nc.gpsimd.tensor_tensor(
                    out=xn[:, DVE_XN:J, 0:D],
                    in0=x_s[:, DVE_XN:J, :],
                    in1=sv[:, DVE_XN:J]
                    .unsqueeze(2)
                    .broadcast_to([128, J - DVE_XN, D]),
                    op=ALU.mult,
                )
            return xn

        def back(n, xn):
            negM = stats.tile([128, J], bf16, tag="negM")
            g = gpool.tile([128, J, K], bf16, tag="g")
            xnt = xtpool.tile([128, J, 128], bf16, tag="xnt")
            pv = pvpool.tile([K, D + 1], f32, tag="pv")

            for h in range(J // HCH):
                # S6/S7: PE transpose xn -> psum (bf16); evacuate via DMA
                for t2 in range(HCH // TCH):
                    jb = h * HCH + t2 * TCH
                    pt = ptpool.tile([128, TCH * 128], bf16, tag="pt")
                    for jj in range(TCH):
                        nc.tensor.transpose(
                            pt[:, jj * 128 : (jj + 1) * 128],
                            xn[:, jb + jj, 0:D],
                            ident,
                        )
                    if t2 == 0:
                        nc.vector.tensor_copy(
                            out=xnt[:, jb : jb + TCH, :],
                            in_=pt.rearrange("p (c d) -> p c d", c=TCH),
                        )
                    else:
                        nc.scalar.copy(
                            out=xnt[:, jb : jb + TCH, :],
                            in_=pt.rearrange("p (c d) -> p c d", c=TCH),
                        )

                # S8: logits pr[tok, HCH*K] = xnt.T @ wt + (A+B) bias rows
                pr = prpool.tile([128, HCH * K], f32, tag="pr")
                for jl in range(HCH):
                    nc.tensor.matmul(
                        pr[:, jl * K : (jl + 1) * K],
                        xnt[:, h * HCH + jl, :],
                        wt_s,
                        start=(jl % ECH == 0),
                        stop=False,
                    )
                for bq in range(HCH // ECH):
                    # closes the bank's group so the M-reduce may read it
                    nc.tensor.matmul(
                        pr[:, bq * ECH * K : (bq + 1) * ECH * K],
                        ones2,
                        ab_s,
                        start=False,
                        stop=True,
                    )
                # S9: negM = -max_k (per token, per tile) in bf16, whole half
                nc.vector.tensor_reduce(
                    out=negM[:, h * HCH : (h + 1) * HCH],
                    in_=pr.rearrange("p (c k) -> p c k", c=HCH),
                    axis=mybir.AxisListType.X,
                    op=ALU.max,
                    negate=True,
                )
                # S10: Mrow[jl, tok] = transpose(negM half) (PE) -> SBUF (ACT)
                pm = pmpool.tile([HCH, 128], bf16, tag="pm")
                nc.tensor.transpose(pm, negM[:, h * HCH : (h + 1) * HCH], ident)
                mrow = stats.tile([HCH, 128], bf16, tag="mrow", bufs=4)
                nc.scalar.copy(out=mrow, in_=pm)
                # S11: pr += Mrow ⊗ sel (per-token max shift, PE)
                for bq in range(HCH // ECH):
                    nc.tensor.matmul(
                        pr[:, bq * ECH * K : (bq + 1) * ECH * K],
                        mrow,
                        sel_s[:, bq * ECH * K : (bq + 1) * ECH * K],
                        start=False,
                        stop=True,
                        skip_group_check=True,
                    )
                # S12: g = Exp(pr) per bank (ACT, const scale/bias)
                for bq in range(HCH // ECH):
                    nc.scalar.activation(
                        out=g[
                            :, h * HCH + bq * ECH : h * HCH + (bq + 1) * ECH, :
                        ],
                        in_=pr[:, bq * ECH * K : (bq + 1) * ECH * K].rearrange(
                            "p (c k) -> p c k", c=ECH
                        ),
                        func=AF.Exp,
                    )
                # S13: Z = sum_k g ; S14: rho = mask / Z   (per half)
                gh = g[:, h * HCH : (h + 1) * HCH, :]
                Zh = stats.tile([128, HCH], bf16, tag="Z", bufs=4)
                gz1 = stats.tile([128, HCH, 32], bf16, tag="gz1", bufs=4)
                with nc.allow_low_precision(reason="Z in [1,64], bf16 0.4%"):
                    nc.vector.tensor_tensor(
                        out=gz1,
                        in0=gh[:, :, 0:32],
                        in1=gh[:, :, 32:64],
                        op=ALU.add,
                    )
                    nc.vector.tensor_reduce(
                        out=Zh, in_=gz1, axis=mybir.AxisListType.X, op=ALU.add
                    )
                zr = stats.tile([128, HCH], f32, tag="zr", bufs=4)
                nc.vector.reciprocal(out=zr, in_=Zh)
                rho = stats.tile([128, HCH], bf16, tag="rho", bufs=4)
                nc.vector.tensor_tensor(
                    out=rho,
                    in0=zr,
                    in1=mask_s[:, n, h * HCH : (h + 1) * HCH],
                    op=ALU.mult,
                )
                # S15: a = g * rho (in place, GpSimd, broadcast rho along k)
                nc.gpsimd.tensor_tensor(
                    out=gh,
                    in0=gh,
                    in1=rho.unsqueeze(2).broadcast_to([128, HCH, K]),
                    op=ALU.mult,
                )
                # S16: vlad_raw += a.T @ [xn | 1] (col 128 = colsum(a))
                for jl in range(HCH):
                    j = h * HCH + jl
                    nc.tensor.matmul(
                        pv,
                        g[:, j, :],
                        xn[:, j, 0 : D + 1],
                        start=(j == 0),
                        stop=(j == J - 1),
                    )
            # S17: stage vlad + colsum to SBUF
            nc.scalar.copy(out=vst[:, n, :], in_=pv)

        # ---- skewed pipeline emission: Fa(n+2) | Fb(n+1) | B(n) ----
        fa = {0: front_a(0)}
        if NS > 1:
            fa[1] = front_a(1)
        fb = {0: front_b(0, *fa.pop(0))}
        for n in range(NS):
            if n + 2 < NS:
                fa[n + 2] = front_a(n + 2)
            if n + 1 < NS:
                fb[n + 1] = front_b(n + 1, *fa.pop(n + 1))
            back(n, fb.pop(n))

        # ---- epilogue over all samples: [64, NS, *] ----
        negcs = stats.tile([K, NS], f32, tag="negcs")
        nc.vector.tensor_scalar(
            out=negcs, in0=vst[:, :, 128], scalar1=-1.0, scalar2=None, op0=ALU.mult
        )
        vl = singles.tile([K, NS, D], f32)
        for n in range(NS):
            # vlad = first_term - colsum*cent
            nc.vector.scalar_tensor_tensor(
                out=vl[:, n, :],
                in0=cent_s,
                scalar=negcs[:, n : n + 1],
                in1=vst[:, n, 0:D],
                op0=ALU.mult,
                op1=ALU.add,
            )
        v2 = singles.tile([K, NS, D], f32)
        nc.vector.tensor_tensor(out=v2, in0=vl, in1=vl, op=ALU.mult)
        ssv = stats.tile([K, NS], f32, tag="ssv")
        nc.vector.tensor_reduce(
            out=ssv, in_=v2, axis=mybir.AxisListType.X, op=ALU.add
        )
        # rv = 1/max(||row||, 1e-12)  (clamp ss at 1e-24; recip + sqrt)
        nc.vector.tensor_scalar(
            out=ssv, in0=ssv, scalar1=1e-24, scalar2=None, op0=ALU.max
        )
        rsv = stats.tile([K, NS], f32, tag="rsv")
        nc.vector.reciprocal(out=rsv, in_=ssv)
        rv = stats.tile([K, NS], f32, tag="rv")
        nc.scalar.activation(out=rv, in_=rsv, func=AF.Sqrt)
        # global: gs[n] = sum_k ssv*rv^2 (cross-partition on GpSimd)
        u1 = stats.tile([K, NS], f32, tag="u1")
        nc.vector.tensor_tensor(out=u1, in0=ssv, in1=rv, op=ALU.mult)
        nc.vector.tensor_tensor(out=u1, in0=u1, in1=rv, op=ALU.mult)
        gs = stats.tile([K, NS], f32, tag="gs")
        nc.gpsimd.partition_all_reduce(
            gs, u1, channels=K, reduce_op=bass_isa.ReduceOp.add
        )
        nc.vector.tensor_scalar(
            out=gs, in0=gs, scalar1=1e-24, scalar2=None, op0=ALU.max
        )
        rgs = stats.tile([K, NS], f32, tag="rgs")
        nc.vector.reciprocal(out=rgs, in_=gs)
        rg = stats.tile([K, NS], f32, tag="rg")
        nc.scalar.activation(out=rg, in_=rgs, func=AF.Sqrt)
        fsc = stats.tile([K, NS], f32, tag="fsc")
        nc.vector.tensor_tensor(out=fsc, in0=rv, in1=rg, op=ALU.mult)
        vo = singles.tile([K, NS, D], f32)
        for n in range(NS):
            nc.vector.tensor_scalar(
                out=vo[:, n, :],
                in0=vl[:, n, :],
                scalar1=fsc[:, n : n + 1],
                scalar2=None,
                op0=ALU.mult,
            )
        # one DMA out: [k, n, d] -> out[n, (k d)]
        nc.sync.dma_start(
            out=out_d[:, :].rearrange("n (k d) -> k n d", k=K), in_=vo
        )


def kernel(x, centroids, weight, bias, masks):
    x = np.ascontiguousarray(x, dtype=np.float32)
    centroids = np.asarray(centroids, dtype=np.float32)
    weight = np.asarray(weight, dtype=np.float32)
    bias = np.asarray(bias, dtype=np.float32)
    masks = np.ascontiguousarray(masks, dtype=np.float32)

    if "nc" not in _CACHE:
        _CACHE["nc"] = _build_nc()
    nc = _CACHE["nc"]

    wt = np.ascontiguousarray(weight.T).astype(ml_dtypes.bfloat16)  # [D, K]
    # exact bias fold: lnE = b - max b + 60 split into bf16 A + bf16 B
    lnE = (bias - bias.max() + 60.0).astype(np.float32)
    A = lnE.astype(ml_dtypes.bfloat16)
    B = (lnE - A.astype(np.float32)).astype(ml_dtypes.bfloat16)
    ab = np.stack([np.tile(A, ECH), np.tile(B, ECH)])  # [2, ECH*K]
    ab = np.ascontiguousarray(ab)
    sel = np.zeros((HCH, HCH * K), dtype=ml_dtypes.bfloat16)
    for j in range(HCH):
        sel[j, j * K : (j + 1) * K] = 1.0
    ident = np.eye(128, dtype=np.float32).astype(ml_dtypes.bfloat16)

    in_maps = []
    for c in range(NCORES):
        sl = slice(c * NS, (c + 1) * NS)
        mcore = masks[sl].reshape(NS, 128, J).transpose(1, 0, 2)  # [128, NS, J]
        in_maps.append(
            {
                "x": x[sl],
                "wt": wt,
                "ab": ab,
                "sel": sel,
                "cent": centroids,
                "ident": ident,
                "masks": np.ascontiguousarray(mcore),
            }
        )

    res = run_bass_kernel_spmd(nc, in_maps, core_ids=list(range(NCORES)))
    _CACHE["last_res"] = res
    outs = [res.results[c]["out"] for c in range(NCORES)]
    return np.concatenate(outs, axis=0).reshape(N, K * D).astype(np.float32)


# revision 55
# speedup vs baseline: 1.3313x; 1.0676x over previous
"""NetVLAD forward kernel for Trainium2 (8 NeuronCores, data-parallel over batch).

Shapes (hardcoded): x (64, 4096, 128) f32, centroids/weight (64, 128), bias (64),
masks (64, 4096). Output (64, 8192) f32. Each core handles 8 samples.

Math (per sample):
  xn = x / ||x||_row                      (row L2 norm over d)
  logits = xn @ w.T + b ; a = softmax_k(logits) * mask
  vlad[k,d] = sum_c a*xn - (sum_c a) * cent[k,d] ; intra + global L2 norm.

Device algorithm (all matmuls bf16, big-instruction softmax), emitted as a
3-stage software pipeline Fa(n+2) | Fb(n+1) | B(n) to avoid per-engine
head-of-line blocking across samples:
  Fa: DMA x; xsq = Square(x) bf16 (ACT)
  Fb: ss via bf16 pairwise folds + reduce (DVE); s = sqrt(1/ss);
      xn = x * s_bcast -> bf16 [tokens, d] (+ones aug col)  (DVE)
  B, per 16-tile half (double-buffered 2-bank PSUM):
    xnt = PE-transpose(xn) -> bf16 PSUM, evacuated by DVE/ACT copies
    pr  = xnt.T @ wt (+ exact bias via 2 bf16 A/B rows ⊗ ones; PE accum,
          one accumulation group per 2KB PSUM bank)
    negM = -max_k pr (DVE) -> PE-transpose -> Mrow; pr += Mrow ⊗ sel (PE)
    g   = Exp(pr) per 512-col bank (ACT, const scale/bias)
    Z   = reduce_k g (DVE); a = g * (mask/Z)_bcast (GpSimd, in place)
    vlad_raw[k, 0:128] (+ colsum col 128) += a.T @ [xn | 1]  (PE)
Epilogue (per core): vlad = first - colsum*cent, intra + global L2 norm.
"""

import numpy as np
import ml_dtypes

import concourse.bass as bass
import concourse.bass_isa as bass_isa
import concourse.mybir as mybir
import concourse.tile as tile
from concourse import bacc
from concourse.bass_utils import run_bass_kernel_spmd

f32 = mybir.dt.float32
bf16 = mybir.dt.bfloat16
AF = mybir.ActivationFunctionType
ALU = mybir.AluOpType

N, C, D, K = 64, 4096, 128, 64
NCORES = 8
NS = N // NCORES          # samples per core
J = C // 128              # 32 token-tiles per sample
TCH = 8                   # transpose tiles per PSUM chunk (1 bank, bf16)
ECH = 8                   # logits tiles per PSUM bank (512 f32)
HCH = 16                  # tiles per half (pr double-buffer unit)
XW = 130                  # xn free width: 128 data + 1 ones-aug (+1 pad)
DVE_XN = 32               # xn tiles computed on DVE (rest on GpSimd)

_CACHE = {}


def _build_nc():
    nc = bacc.Bacc("TRN2", target_bir_lowering=False)
    x_d = nc.dram_tensor("x", [NS, C, D], f32, kind="ExternalInput")
    wt_d = nc.dram_tensor("wt", [D, K], bf16, kind="ExternalInput")
    ab_d = nc.dram_tensor("ab", [2, ECH * K], bf16, kind="ExternalInput")
    sel_d = nc.dram_tensor("sel", [HCH, HCH * K], bf16, kind="ExternalInput")
    cent_d = nc.dram_tensor("cent", [K, D], f32, kind="ExternalInput")
    ident_d = nc.dram_tensor("ident", [128, 128], bf16, kind="ExternalInput")
    mask_d = nc.dram_tensor("masks", [128, NS, J], f32, kind="ExternalInput")
    out_d = nc.dram_tensor("out", [NS, K * D], f32, kind="ExternalOutput")

    with tile.TileContext(nc) as tc:
        _netvlad(tc, x_d, wt_d, ab_d, sel_d, cent_d, ident_d, mask_d, out_d)
    nc.compile()
    return nc


def _netvlad(tc, x_d, wt_d, ab_d, sel_d, cent_d, ident_d, mask_d, out_d):
    nc = tc.nc
    from contextlib import ExitStack

    with ExitStack() as ctx:
        singles = ctx.enter_context(tc.tile_pool(name="singles", bufs=1))
        xpool = ctx.enter_context(tc.tile_pool(name="xp", bufs=2))
        sqpool = ctx.enter_context(tc.tile_pool(name="sqp", bufs=2))
        xnpool = ctx.enter_context(tc.tile_pool(name="xnp", bufs=3))
        xtpool = ctx.enter_context(tc.tile_pool(name="xtp", bufs=3))
        gpool = ctx.enter_context(tc.tile_pool(name="gp", bufs=3))
        stats = ctx.enter_context(tc.tile_pool(name="stats", bufs=2))
        ptpool = ctx.enter_context(tc.tile_pool(name="ptp", bufs=2, space="PSUM"))
        prpool = ctx.enter_context(tc.tile_pool(name="prp", bufs=2, space="PSUM"))
        pmpool = ctx.enter_context(tc.tile_pool(name="pmp", bufs=1, space="PSUM"))
        pvpool = ctx.enter_context(tc.tile_pool(name="pvp", bufs=1, space="PSUM"))

        # ---- constants ----
        wt_s = singles.tile([D, K], bf16)
        nc.sync.dma_start(out=wt_s, in_=wt_d[:, :])
        ab_s = singles.tile([2, ECH * K], bf16)
        nc.sync.dma_start(out=ab_s, in_=ab_d[:, :])
        sel_s = singles.tile([HCH, HCH * K], bf16)
        nc.sync.dma_start(out=sel_s, in_=sel_d[:, :])
        cent_s = singles.tile([K, D], f32)
        nc.sync.dma_start(out=cent_s, in_=cent_d[:, :])
        ident = singles.tile([128, 128], bf16)
        nc.sync.dma_start(out=ident, in_=ident_d[:, :])
        mask_s = singles.tile([128, NS, J], f32)
        nc.sync.dma_start(out=mask_s, in_=mask_d[:, :, :])
        ones2 = singles.tile([2, 128], bf16)
        nc.vector.memset(ones2, 1.0)
        # staging for per-sample vlad rows + colsum (64 partitions)
        vst = singles.tile([K, NS, 129], f32)

        def front_a(n):
            """DMA the sample in and square it (ACT)."""
            x_s = xpool.tile([128, J, D], f32, tag="x", bufs=3)
            nc.sync.dma_start(
                out=x_s, in_=x_d[n, :, :].rearrange("(p j) d -> p j d", j=J)
            )
            xsq = sqpool.tile([128, J, D], bf16, tag="xsq", bufs=3)
            nc.scalar.activation(out=xsq, in_=x_s, func=AF.Square)
            return x_s, xsq

        def front_b(n, x_s, xsq):
            """ss folds -> s -> xn (DVE/GpSimd/ACT)."""
            xf1 = sqpool.tile([128, J, 64], bf16, tag="xf1")
            ss = stats.tile([128, J], bf16, tag="ss")
            with nc.allow_low_precision(reason="ss bf16: 0.4% rel, gate 2e-2"):
                nc.vector.tensor_tensor(
                    out=xf1,
                    in0=xsq[:, :, 0:64],
                    in1=xsq[:, :, 64:128],
                    op=ALU.add,
                )
                xf2 = sqpool.tile([128, J, 32], bf16, tag="xf2")
                nc.vector.tensor_tensor(
                    out=xf2, in0=xf1[:, :, 0:32], in1=xf1[:, :, 32:64], op=ALU.add
                )
                nc.vector.tensor_reduce(
                    out=ss, in_=xf2, axis=mybir.AxisListType.X, op=ALU.add
                )
            rss = stats.tile([128, J], f32, tag="rss")
            nc.vector.reciprocal(out=rss, in_=ss)
            sv = stats.tile([128, J], f32, tag="sv")
            nc.scalar.activation(out=sv, in_=rss, func=AF.Sqrt)
            # xn = x * s (broadcast s along d), bf16, + ones aug col
            xn = xnpool.tile([128, J, XW], bf16, tag="xn")
            nc.gpsimd.memset(xn[:, :, D], 1.0)
            nc.vector.tensor_tensor(
                out=xn[:, 0:DVE_XN, 0:D],
                in0=x_s[:, 0:DVE_XN, :],
                in1=sv[:, 0:DVE_XN].unsqueeze(2).broadcast_to([128, DVE_XN, D]),
                op=ALU.mult,
            )
            if DVE_XN < J:
                nc.gpsimd.tensor_tensor(
                    out=xn[:, DVE_XN:J, 0:D],
                    in0=x_s[:, DVE_XN:J, :],
                    in1=sv[:, DVE_XN:J]
                    .unsqueeze(2)
                    .broadcast_to([128, J - DVE_XN, D]),
                    op=ALU.mult,
                )
            return xn

        def back(n, xn):
            negM = stats.tile([128, J], bf16, tag="negM")
            g = gpool.tile([128, J, K], bf16, tag="g")
            xnt = xtpool.tile([128, J, 128], bf16, tag="xnt")
            pv = pvpool.tile([K, D + 1], f32, tag="pv")

            for h in range(J // HCH):
                # S6/S7: PE transpose xn -> psum (bf16); evacuate via DMA
                for t2 in range(HCH // TCH):
                    jb = h * HCH + t2 * TCH
                    pt = ptpool.tile([128, TCH * 128], bf16, tag="pt")
                    for jj in range(TCH):
                        nc.tensor.transpose(
                            pt[:, jj * 128 : (jj + 1) * 128],
                            xn[:, jb + jj, 0:D],
                            ident,
                        )
                    nc.scalar.copy(
                        out=xnt[:, jb : jb + TCH, :],
                        in_=pt.rearrange("p (c d) -> p c d", c=TCH),
                    )

                # S8: logits pr[tok, HCH*K] = xnt.T @ wt + (A+B) bias rows
                pr = prpool.tile([128, HCH * K], f32, tag="pr")
                for jl in range(HCH):
                    nc.tensor.matmul(
                        pr[:, jl * K : (jl + 1) * K],
                        xnt[:, h * HCH + jl, :],
                        wt_s,
                        start=(jl % ECH == 0),
                        stop=False,
                    )
                for bq in range(HCH // ECH):
                    # closes the bank's group so the M-reduce may read it
                    nc.tensor.matmul(
                        pr[:, bq * ECH * K : (bq + 1) * ECH * K],
                        ones2,
                        ab_s,
                        start=False,
                        stop=True,
                    )
                # S9: negM = -max_k (per token, per tile) in bf16, whole half
                nc.vector.tensor_reduce(
                    out=negM[:, h * HCH : (h + 1) * HCH],
                    in_=pr.rearrange("p (c k) -> p c k", c=HCH),
                    axis=mybir.AxisListType.X,
                    op=ALU.max,
                    negate=True,
                )
                # S10: Mrow[jl, tok] = transpose(negM half) (PE) -> SBUF (ACT)
                pm = pmpool.tile([HCH, 128], bf16, tag="pm")
                nc.tensor.transpose(pm, negM[:, h * HCH : (h + 1) * HCH], ident)
                mrow = stats.tile([HCH, 128], bf16, tag="mrow", bufs=4)
                nc.scalar.copy(out=mrow, in_=pm)
                # S11: pr += Mrow ⊗ sel (per-token max shift, PE)
                for bq in range(HCH // ECH):
                    nc.tensor.matmul(
                        pr[:, bq * ECH * K : (bq + 1) * ECH * K],
                        mrow,
                        sel_s[:, bq * ECH * K : (bq + 1) * ECH * K],
                        start=False,
                        stop=True,
                        skip_group_check=True,
                    )
                # S12: g = Exp(pr) per bank (ACT, const scale/bias)
                for bq in range(HCH // ECH):
                    nc.scalar.activation(
                        out=g[
                            :, h * HCH + bq * ECH : h * HCH + (bq + 1) * ECH, :
                        ],
                        in_=pr[:, bq * ECH * K : (bq + 1) * ECH * K].rearrange(
                            "p (c k) -> p c k", c=ECH
                        ),
                        func=AF.Exp,
                    )
                # S13: Z = sum_k g ; S14: rho = mask / Z   (per half)
                gh = g[:, h * HCH : (h + 1) * HCH, :]
                Zh = stats.tile([128, HCH], bf16, tag="Z", bufs=4)
                gz1 = stats.tile([128, HCH, 32], bf16, tag="gz1", bufs=4)
                with nc.allow_low_precision(reason="Z in [1,64], bf16 0.4%"):
                    nc.vector.tensor_tensor(
                        out=gz1,
                        in0=gh[:, :, 0:32],
                        in1=gh[:, :, 32:64],
                        op=ALU.add,
                    )
                    nc.vector.tensor_reduce(
                        out=Zh, in_=gz1, axis=mybir.AxisListType.X, op=ALU.add
                    )
                zr = stats.tile([128, HCH], f32, tag="zr", bufs=4)
                nc.vector.reciprocal(out=zr, in_=Zh)
                rho = stats.tile([128, HCH], bf16, tag="rho", bufs=4)
                nc.vector.tensor_tensor(
                    out=rho,
                    in0=zr,
                    in1=mask_s[:, n, h * HCH : (h + 1) * HCH],
                    op=ALU.mult,
                )
                # S15: a = g * rho (in place, GpSimd, broadcast rho along k)
                nc.gpsimd.tensor_tensor(
                    out=gh,
                    in0=gh,
                    in1=rho.unsqueeze(2).broadcast_to([128, HCH, K]),
                    op=ALU.mult,
                )
                # S16: vlad_raw += a.T @ [xn | 1] (col 128 = colsum(a))
                for jl in range(HCH):
                    j = h * HCH + jl
                    nc.tensor.matmul(
                        pv,
                        g[:, j, :],
                        xn[:, j, 0 : D + 1],
                        start=(j == 0),
                        stop=(j == J - 1),
                    )
            # S17: stage vlad + colsum to SBUF
            nc.scalar.copy(out=vst[:, n, :], in_=pv)

        # ---- skewed pipeline emission: Fa(n+2) | Fb(n+1) | B(n) ----
        fa = {0: front_a(0)}
        if NS > 1:
            fa[1] = front_a(1)
        fb = {0: front_b(0, *fa.pop(0))}
        for n in range(NS):
            if n + 2 < NS:
                fa[n + 2] = front_a(n + 2)
            if n + 1 < NS:
                fb[n + 1] = front_b(n + 1, *fa.pop(n + 1))
            back(n, fb.pop(n))

        # ---- epilogue over all samples: [64, NS, *] ----
        negcs = stats.tile([K, NS], f32, tag="negcs")
        nc.vector.tensor_scalar(
            out=negcs, in0=vst[:, :, 128], scalar1=-1.0, scalar2=None, op0=ALU.mult
        )
        vl = singles.tile([K, NS, D], f32)
        for n in range(NS):
            # vlad = first_term - colsum*cent
            nc.vector.scalar_tensor_tensor(
                out=vl[:, n, :],
                in0=cent_s,
                scalar=negcs[:, n : n + 1],
                in1=vst[:, n, 0:D],
                op0=ALU.mult,
                op1=ALU.add,
            )
        v2 = singles.tile([K, NS, D], f32)
        nc.vector.tensor_tensor(out=v2, in0=vl, in1=vl, op=ALU.mult)
        ssv = stats.tile([K, NS], f32, tag="ssv")
        nc.vector.tensor_reduce(
            out=ssv, in_=v2, axis=mybir.AxisListType.X, op=ALU.add
        )
        # rv = 1/max(||row||, 1e-12)  (clamp ss at 1e-24; recip + sqrt)
        nc.vector.tensor_scalar(
            out=ssv, in0=ssv, scalar1=1e-24, scalar2=None, op0=ALU.max
        )
        rsv = stats.tile([K, NS], f32, tag="rsv")
        nc.vector.reciprocal(out=rsv, in_=ssv)
        rv = stats.tile([K, NS], f32, tag="rv")
        nc.scalar.activation(out=rv, in_=rsv, func=AF.Sqrt)
        # global: gs[n] = sum_k ssv*rv^2 (cross-partition on GpSimd)
        u1 = stats.tile([K, NS], f32, tag="u1")
        nc.vector.tensor_tensor(out=u1, in0=ssv, in1=rv, op=ALU.mult)
        nc.vector.tensor_tensor(out=u1, in0=u1, in1=rv, op=ALU.mult)
        gs = stats.tile([K, NS], f32, tag="gs")
        nc.gpsimd.partition_all_reduce(
            gs, u1, channels=K, reduce_op=bass_isa.ReduceOp.add
        )
        nc.vector.tensor_scalar(
            out=gs, in0=gs, scalar1=1e-24, scalar2=None, op0=ALU.max
        )
        rgs = stats.tile([K, NS], f32, tag="rgs")
        nc.vector.reciprocal(out=rgs, in_=gs)
        rg = stats.tile([K, NS], f32, tag="rg")
        nc.scalar.activation(out=rg, in_=rgs, func=AF.Sqrt)
        fsc = stats.tile([K, NS], f32, tag="fsc")
        nc.vector.tensor_tensor(out=fsc, in0=rv, in1=rg, op=ALU.mult)
        vo = singles.tile([K, NS, D], f32)
        for n in range(NS):
            nc.vector.tensor_scalar(
                out=vo[:, n, :],
                in0=vl[:, n, :],
                scalar1=fsc[:, n : n + 1],
                scalar2=None,
                op0=ALU.mult,
            )
        # one DMA out: [k, n, d] -> out[n, (k d)]
        nc.sync.dma_start(
            out=out_d[:, :].rearrange("n (k d) -> k n d", k=K), in_=vo
        )


def kernel(x, centroids, weight, bias, masks):
    x = np.ascontiguousarray(x, dtype=np.float32)
    centroids = np.asarray(centroids, dtype=np.float32)
    weight = np.asarray(weight, dtype=np.float32)
    bias = np.asarray(bias, dtype=np.float32)
    masks = np.ascontiguousarray(masks, dtype=np.float32)

    if "nc" not in _CACHE:
        _CACHE["nc"] = _build_nc()
    nc = _CACHE["nc"]

    wt = np.ascontiguousarray(weight.T).astype(ml_dtypes.bfloat16)  # [D, K]
    # exact bias fold: lnE = b - max b + 60 split into bf16 A + bf16 B
    lnE = (bias - bias.max() + 60.0).astype(np.float32)
    A = lnE.astype(ml_dtypes.bfloat16)
    B = (lnE - A.astype(np.float32)).astype(ml_dtypes.bfloat16)
    ab = np.stack([np.tile(A, ECH), np.tile(B, ECH)])  # [2, ECH*K]
    ab = np.ascontiguousarray(ab)
    sel = np.zeros((HCH, HCH * K), dtype=ml_dtypes.bfloat16)
    for j in range(HCH):
        sel[j, j * K : (j + 1) * K] = 1.0
    ident = np.eye(128, dtype=np.float32).astype(ml_dtypes.bfloat16)

    in_maps = []
    for c in range(NCORES):
        sl = slice(c * NS, (c + 1) * NS)
        mcore = masks[sl].reshape(NS, 128, J).transpose(1, 0, 2)  # [128, NS, J]
        in_maps.append(
            {
                "x": x[sl],
                "wt": wt,
                "ab": ab,
                "sel": sel,
                "cent": centroids,
                "ident": ident,
                "masks": np.ascontiguousarray(mcore),
            }
        )

    res = run_bass_kernel_spmd(nc, in_maps, core_ids=list(range(NCORES)))
    _CACHE["last_res"] = res
    outs = [res.results[c]["out"] for c in range(NCORES)]
    return np.concatenate(outs, axis=0).reshape(N, K * D).astype(np.float32)
